# revision 13
# baseline (speedup 1.0000x reference)
"""Trainium2 Bass kernel for DifferentiableDLT (batched weighted-DLT homography fit).

Contract: kernel(**inputs) takes FULL inputs
    flow (64, 2, 320, 576) f32, mask (64, 1, 320, 576) f32, img_h, img_w
and returns the FULL output (64, 3, 3) f32.

Strategy (pure data parallel, 8 batches/core x 8 cores):
  The 1024 sample points form a fixed separable 32x32 grid, so bilinear
  sampling touches only 32 pair-rows (y0, y0+1) per image, and the y-steps
  follow a 9,9,9,10 pattern whose irregular step always lands on a k%4
  boundary.  Host-side we concatenate flow and mask into one (8,3,H,W)
  tensor so a single pipeline handles both.  Per core:
    1. 9 direct strided HWDGE dma_start pushes (no gather / index tables /
       gpsimd descriptor generation) land all 32 pair-rows x 24 images into
       SBUF as tFM[96=(kd4,c3,b8)][s8][1152], k = 4*s + kd.
    2. Column-select the 64 needed columns per pair-row with 9 uniform-
       stride-run copies per (half, c2), split across Vector and GpSimd.
    3. Bilinear lerp in y then x (broadcast weight tiles), per s-half so
       compute overlaps the tail of the DMA.
    4. Per half, one PE matmul transposes points onto partitions, fused
       with image-coord scaling (diag) and grid offsets (rank-5 factor).
    5. Hartley-normalize dst points; build weighted feature products D =
       [w, w*p, w*q, w*r2]; moments M = C^T @ D via 8 accumulating matmuls.
    6. Assemble the 8x9 augmented normal equations per batch (batch on
       partitions) and solve with unpivoted Gauss-Jordan (SPD + eps*I).
    7. Denormalize H, sign/scale fix, support gate, DMA out (8,3,3).
"""

import dataclasses
import math
import numpy as np

import concourse.bass as bass
import concourse.bacc as bacc
import concourse.mybir as mybir
from concourse import tile, library_config
from concourse import bass_utils

F32 = mybir.dt.float32
ALU = mybir.AluOpType
ACTF = mybir.ActivationFunctionType

NCORES = 8
BPC = 8          # batches per core
HF, WF = 320, 576
NG = 32          # grid is NG x NG points
NPTS = NG * NG
EPS = 1e-6
NC_IM = 3        # flow-x, flow-y, mask channels in the fused input
NIM = NC_IM * BPC   # 24 images per core
PW = 2 * WF      # pair-row width (rows y0, y0+1 contiguous)

# column layout offsets inside the packed constant tensors
CA_WY, CA_WX, CA_W = 0, 8, 40
CB_SXYD, CB_G5, CB_GR5, CB_C6, CB_IDN, CB_EQ, CB_W = 0, 96, 224, 320, 368, 440, 728

# ---------------------------------------------------------------------------
# host-side constant computation
# ---------------------------------------------------------------------------


def _grid_1d(size, n):
    m = int(size * 0.05)
    return np.linspace(m, size - m - 1, n, dtype=np.float32)


def _segments(x0):
    """Maximal uniform-step segments (start, len, step) covering x0."""
    segs = []
    i = 0
    n = len(x0)
    while i < n:
        if i == n - 1:
            segs.append((i, 1, 1))
            break
        st = x0[i + 1] - x0[i]
        j = i + 1
        while j + 1 < n and x0[j + 1] - x0[j] == st:
            j += 1
        segs.append((i, j - i + 1, int(st)))
        i = j + 1
    return segs


class _Consts:
    def __init__(self, img_h, img_w):
        ys = _grid_1d(HF, NG)
        xs = _grid_1d(WF, NG)
        y0 = np.floor(ys).astype(np.int64)
        x0 = np.floor(xs).astype(np.int64)
        wy = ys.astype(np.float64) - y0
        wx = xs.astype(np.float64) - x0
        self.segs = _segments(x0)
        self.x0 = x0
        self.y0 = y0
        self.sx = float(np.float32((img_w - 1) / max(WF - 1, 1)))
        self.sy = float(np.float32((img_h - 1) / max(HF - 1, 1)))

        # ---- row-push plan: k = 4*s + kd; uniform-delta runs within a slot
        self.pushes = []  # (k0, nk, kd0, s, delta)
        for s in range(8):
            ks = y0[4 * s : 4 * s + 4]
            i = 0
            while i < 4:
                j = i + 1
                while j < 4 and ks[j] - ks[j - 1] == ks[i + 1] - ks[i]:
                    j += 1
                d = int(ks[i + 1] - ks[i]) if j - i > 1 else 0
                self.pushes.append((4 * s + i, j - i, i, s, d))
                i = j

        # constant Hartley normalization of the source points (image coords)
        jj = np.arange(NPTS) // NG
        ii = np.arange(NPTS) % NG
        sxi = xs.astype(np.float64)[ii] * self.sx
        syi = ys.astype(np.float64)[jj] * self.sy
        mx, my = sxi.mean(), syi.mean()
        cxs, cys = sxi - mx, syi - my
        s_src = max(np.sqrt(cxs * cxs + cys * cys).mean() / math.sqrt(2.0), 1e-8)
        # per x/y 1-D normalized coords
        u1 = (xs.astype(np.float64) * self.sx - mx) / s_src   # (32,) by i
        v1 = (ys.astype(np.float64) * self.sy - my) / s_src   # (32,) by k
        # T_src = [[1/s,0,-mx/s],[0,1/s,-my/s],[0,0,1]] immediates
        self.a_ts = float(np.float32(1.0 / s_src))
        self.c_ts = float(np.float32(-mx / s_src))
        self.d_ts = float(np.float32(-my / s_src))

        # partition p (96) = kd*24 + c*8 + b
        pp = np.arange(96)
        kd_p = pp // 24
        c_p = (pp % 24) // 8

        # ---- CONSTA [128, 40]: WY [96,8] (wy[4s+kd]), WX [96,32] (wx[i])
        CA = np.zeros((128, CA_W), np.float32)
        CA[:96, 0:8] = wy[4 * np.arange(8)[None, :] + kd_p[:, None]]
        CA[:96, 8:40] = np.tile(wx[None, :], (96, 1))
        self.CA = CA

        # ---- CONSTB [128, CB_W] ----
        CB = np.zeros((128, CB_W), np.float32)
        # SXYD [96,96] diag: c==0 -> sx, c==1 -> sy, c==2 -> 1
        sxy = np.where(c_p == 0, self.sx, np.where(c_p == 1, self.sy, 1.0))
        CB[:96, CB_SXYD : CB_SXYD + 96] = np.eye(96) * sxy[None, :]
        # G5 [5,128]: rows sp: (pl//32==sp); row4: xs[pl%32]
        # duplicated at partition 32 (base partition must match GR5's h block)
        pl = np.arange(128)
        G5 = np.zeros((5, 128), np.float64)
        for sp in range(4):
            G5[sp] = (pl // 32 == sp).astype(np.float64)
        G5[4] = xs.astype(np.float64)[pl % 32]
        CB[:5, CB_G5 : CB_G5 + 128] = G5.astype(np.float32)
        CB[32:37, CB_G5 : CB_G5 + 128] = G5.astype(np.float32)
        # GR5 rows 32h+sp: (c==1)*ys[4*(4h+sp)+kd]*sy ; 32h+4: (c==0)*sx
        # (h=1 block lives at partition 32 -- matmul base-partition rule)
        GR5 = np.zeros((37, 96), np.float64)
        for h in range(2):
            for sp in range(4):
                kk = 4 * (4 * h + sp) + kd_p
                GR5[32 * h + sp] = np.where(
                    c_p == 1, ys.astype(np.float64)[kk] * self.sy, 0.0)
            GR5[32 * h + 4] = np.where(c_p == 0, self.sx, 0.0)
        CB[:37, CB_GR5 : CB_GR5 + 96] = GR5.astype(np.float32)
        # C6 [128, 48]: feats [uu, uv, u, vv, v, 1] at (pl=(s',i), t=(h,kd))
        C6 = np.zeros((128, 48), np.float64)
        spl = pl // 32
        ipl = pl % 32
        for t in range(8):
            h, kd = t // 4, t % 4
            kk = 4 * (4 * h + spl) + kd
            u = u1[ipl]
            v = v1[kk]
            C6[:, 6 * t : 6 * t + 6] = np.stack(
                [u * u, u * v, u, v * v, v, np.ones_like(u)], -1)
        CB[:, CB_C6 : CB_C6 + 48] = C6.astype(np.float32)
        # IDN72
        CB[:72, CB_IDN : CB_IDN + 72] = np.eye(72, dtype=np.float32)
        # EQ [6, 288]
        E = np.zeros((4, 6, 72), np.float64)
        sym = [[0, 1, 2], [1, 3, 4], [2, 4, 5]]
        for r in range(3):
            for c in range(3):
                m = sym[r][c]
                E[0, m, r * 9 + c] += 1
                E[0, m, (r + 3) * 9 + (c + 3)] += 1
        cr = [[0, 1], [1, 3], [2, 4]]
        for q, r0 in ((1, 0), (2, 3)):
            for r in range(3):
                for c2 in range(2):
                    m = cr[r][c2]
                    E[q, m, (r0 + r) * 9 + 6 + c2] += -1
                    E[q, m, (6 + c2) * 9 + (r0 + r)] += -1
            for r, m in ((0, 2), (1, 4), (2, 5)):
                E[q, m, (r0 + r) * 9 + 8] += 1
        rb = [[0, 1], [1, 3]]
        for r in range(2):
            for c2 in range(2):
                E[3, rb[r][c2], (6 + r) * 9 + 6 + c2] += 1
        E[3, 2, 6 * 9 + 8] += -1
        E[3, 4, 7 * 9 + 8] += -1
        CB[:6, CB_EQ : CB_EQ + 288] = np.ascontiguousarray(
            E.transpose(1, 0, 2).reshape(6, 288)).astype(np.float32)
        self.CB = CB


# ---------------------------------------------------------------------------
# device program
# ---------------------------------------------------------------------------


def _build_program(cc: _Consts):
    nc = bacc.Bacc("TRN2", target_bir_lowering=False, debug=False)

    fm = nc.dram_tensor("fm", [NC_IM, BPC, HF, WF], F32, kind="ExternalInput")
    CAd = nc.dram_tensor("CA", [128, CA_W], F32, kind="ExternalInput")
    CBd = nc.dram_tensor("CB", [128, CB_W], F32, kind="ExternalInput")
    Hout = nc.dram_tensor("H", [BPC, 3, 3], F32, kind="ExternalOutput")

    V = nc.vector
    A = nc.scalar
    T = nc.tensor
    G = nc.gpsimd
    S = nc.sync

    fm_flat = fm.ap().rearrange("c b h w -> (c b h w)").unsqueeze(0)

    def push_src(k0, nk, delta):
        off = int(cc.y0[k0]) * WF
        v = fm_flat[:, off : off + 1]
        if nk > 1:
            ap = [[delta * WF, nk], [HF * WF, NIM], [1, PW]]
        else:
            ap = [[HF * WF, NIM], [1, PW]]
        return dataclasses.replace(v, ap=ap)

    with tile.TileContext(nc) as tc:
        with (
            tc.tile_pool(name="sb", bufs=1) as pool,
            tc.tile_pool(name="ps", bufs=1, space="PSUM") as psp,
        ):
            tFM = pool.tile([96, 8, PW], F32)     # [(kd,c,b)][s][pair row]
            CAt = pool.tile([128, CA_W], F32, tag="CA")
            CBt = pool.tile([128, CB_W], F32, tag="CB")

            # ---------------- row pushes + constants (2 HWDGE queues) ------
            def dst_slice(kd0, nk, s):
                return tFM[24 * kd0 : 24 * (kd0 + nk), s : s + 1, :].rearrange(
                    "p o w -> p (o w)")

            # order: slot0 | CONSTA | slot1..: alternate queues
            q = [S, A]
            S.dma_start(dst_slice(cc.pushes[0][2], cc.pushes[0][1],
                                  cc.pushes[0][3]),
                        push_src(cc.pushes[0][0], cc.pushes[0][1],
                                 cc.pushes[0][4]))
            A.dma_start(CAt[:, :], CAd[:])
            qi = 0
            for n, (k0, nk, kd0, s, d) in enumerate(cc.pushes[1:], start=1):
                q[qi].dma_start(dst_slice(kd0, nk, s), push_src(k0, nk, d))
                qi ^= 1
                if n == 3:
                    q[qi].dma_start(CBt[:, :], CBd[:])
                    qi ^= 1

            # small on-chip constants
            IEYE = pool.tile([8, 9], F32, tag="IEYE")
            V.memset(IEYE[:, :], 0.0)
            V.memset(IEYE[:, 0:9:4], 1.0)
            ONESC = pool.tile([128, 1], F32, tag="ONESC")
            V.memset(ONESC[:, :], 1.0 / NPTS)
            ONESR = pool.tile([1, 128], F32, tag="ONESR")
            V.memset(ONESR[:, :], 1.0)
            # prefetch the ACT function table (Sqrt/Abs) off the critical path
            ACTJ = pool.tile([8, 2], F32, tag="ACTJ")
            V.memset(ACTJ[:, :], 1.0)
            A.activation(ACTJ[:, 0:1], ACTJ[:, 1:2], ACTF.Sqrt)
            A.activation(ACTJ[:, 1:2], ACTJ[:, 0:1], ACTF.Abs)

            WY = CAt[0:96, CA_WY : CA_WY + 8]
            WX = CAt[0:96, CA_WX : CA_WX + 32]
            SXYD = CBt[0:96, CB_SXYD : CB_SXYD + 96]
            G5GR = CBt[0:37, :]  # G5/GR5 blocks at partitions 32h
            C6 = CBt[0:128, CB_C6 : CB_C6 + 48]
            IDN = CBt[0:72, CB_IDN : CB_IDN + 72]
            EQ = CBt[0:6, CB_EQ : CB_EQ + 288]

            # ------- column select + bilinear interp, per s-half -------
            GxFM = pool.tile([96, 2, 8, 32, 2], F32)  # [a][s][i][c2]
            VF = pool.tile([96, 8, 32, 2], F32)
            DV = pool.tile([96, 8, 32, 2], F32)
            DX = pool.tile([96, 256], F32)
            sampFM = pool.tile([96, 256], F32)        # [(kd,c,b)][(s,i)]
            psF = psp.tile([128, 192], F32)
            PQs = pool.tile([128, 192], F32)          # [pl=(s',i)][(t,c,b)]
            tFMv = tFM[:, :, :].rearrange("p s (a w) -> p a s w", a=2)

            for h in range(2):
                sl = slice(4 * h, 4 * h + 4)
                eng = 0
                for (i0, L, st) in cc.segs:
                    base = int(cc.x0[i0])
                    for c2 in (0, 1):
                        src_ = tFMv[:, :, sl,
                                    base + c2 : base + c2 + (L - 1) * st + 1 : st]
                        dst = GxFM[:, :, sl, i0 : i0 + L, c2]
                        (V if eng == 0 else G).tensor_copy(dst, src_)
                        eng ^= 1
                # y lerp: V = G0 + (G1-G0)*wy  on [96, (s4,i,c2)=256]
                g0 = GxFM[:, 0, sl, :, :]
                g1 = GxFM[:, 1, sl, :, :]
                dv = DV[:, sl, :, :]
                vf = VF[:, sl, :, :]
                wyb = WY[:, sl].unsqueeze(2).unsqueeze(3).broadcast_to(
                    [96, 4, 32, 2])
                V.tensor_tensor(out=dv, in0=g1, in1=g0, op=ALU.subtract)
                V.tensor_tensor(out=dv, in0=dv, in1=wyb, op=ALU.mult)
                V.tensor_tensor(out=vf, in0=dv, in1=g0, op=ALU.add)
                # x lerp: samp = V0 + (V1-V0)*wx  on [96, (s4,i)=128]
                vv = VF[:, sl, :, :]
                dx = DX[:, 128 * h : 128 * h + 128].rearrange(
                    "p (s i) -> p s i", s=4)
                sm = sampFM[:, 128 * h : 128 * h + 128].rearrange(
                    "p (s i) -> p s i", s=4)
                wxb = WX[:, :].unsqueeze(1).broadcast_to([96, 4, 32])
                V.tensor_tensor(out=dx, in0=vv[:, :, :, 1], in1=vv[:, :, :, 0],
                                op=ALU.subtract)
                V.tensor_tensor(out=dx, in0=dx, in1=wxb, op=ALU.mult)
                V.tensor_tensor(out=sm, in0=dx, in1=vv[:, :, :, 0], op=ALU.add)

                # transpose to points-on-partitions, fused with image-coord
                # scaling and grid offsets
                T.matmul(psF[:, 96 * h : 96 * h + 96],
                         sampFM[:, 128 * h : 128 * h + 128], SXYD,
                         start=True, stop=False)
                T.matmul(psF[:, 96 * h : 96 * h + 96],
                         G5GR[32 * h : 32 * h + 5, CB_G5 : CB_G5 + 128],
                         G5GR[32 * h : 32 * h + 5, CB_GR5 : CB_GR5 + 96],
                         start=False, stop=True)
                V.tensor_copy(PQs[:, 96 * h : 96 * h + 96],
                              psF[:, 96 * h : 96 * h + 96])

            # ---------------- Hartley stats ----------------
            PQv = PQs[:, :].rearrange("p (t c b) -> p t c b", t=8, c=3, b=8)
            psSum = psp.tile([1, 192], F32, tag="pss")
            T.matmul(psSum[:, :], ONESC[:, :], PQs[:, :], start=True, stop=True)
            SRow = pool.tile([1, 192], F32)
            V.tensor_copy(SRow[:, :], psSum[:, :])
            MRow = pool.tile([1, 24], F32)   # [(c3,b8)] means over points
            V.tensor_reduce(
                out=MRow[:, :].unsqueeze(1),
                in_=SRow[:, :].rearrange("o (t cb) -> o cb t", t=8, cb=24),
                axis=mybir.AxisListType.X,
                op=ALU.add,
            )
            psMB = psp.tile([128, 24], F32, tag="psb")
            T.matmul(psMB[:, :], ONESR[:, :], MRow[:, :], start=True, stop=True)
            MB = pool.tile([128, 24], F32)
            V.tensor_copy(MB[:, :], psMB[:, :])

            CXY = pool.tile([128, 128], F32)  # centered dst [t][c2][b]
            SQ = pool.tile([128, 128], F32)
            R2 = pool.tile([128, 64], F32)
            SQR = pool.tile([128, 64], F32)
            mbv = MB[:, :].rearrange("p (c b) -> p c b", c=3)[:, 0:2, :]
            V.tensor_tensor(
                out=CXY[:, :].rearrange("p (t c b) -> p t c b", t=8, c=2, b=8),
                in0=PQv[:, :, 0:2, :],
                in1=mbv.unsqueeze(1).broadcast_to([128, 8, 2, 8]),
                op=ALU.subtract)
            V.tensor_tensor(out=SQ[:, :], in0=CXY[:, :], in1=CXY[:, :],
                            op=ALU.mult)
            sq3 = SQ[:, :].rearrange("p (t c b) -> p c t b", t=8, c=2, b=8)
            V.tensor_tensor(out=R2[:, :].rearrange("p (t b) -> p t b", t=8),
                            in0=sq3[:, 0, :, :], in1=sq3[:, 1, :, :], op=ALU.add)
            A.activation(SQR[:, :], R2[:, :], ACTF.Sqrt)
            psSq = psp.tile([1, 64], F32, tag="pss")
            T.matmul(psSq[:, :], ONESC[:, :], SQR[:, :], start=True, stop=True)
            SqRow = pool.tile([1, 64], F32)
            V.tensor_copy(SqRow[:, :], psSq[:, :])
            sRow = pool.tile([1, 8], F32)
            V.tensor_reduce(
                out=sRow[:, :].unsqueeze(1),
                in_=SqRow[:, :].rearrange("o (t b) -> o b t", t=8),
                axis=mybir.AxisListType.X,
                op=ALU.add,
            )
            V.tensor_scalar(out=sRow[:, :], in0=sRow[:, :],
                            scalar1=1.0 / math.sqrt(2.0), op0=ALU.mult,
                            scalar2=1e-8, op1=ALU.max)
            IR24 = pool.tile([1, 24], F32)   # [inv | inv | inv^2]
            V.reciprocal(IR24[:, 0:8], sRow[:, :])
            V.tensor_copy(IR24[:, 8:16], IR24[:, 0:8])
            V.tensor_tensor(out=IR24[:, 16:24], in0=IR24[:, 0:8],
                            in1=IR24[:, 0:8], op=ALU.mult)

            # ------ D = [w, w*cx, w*cy, w*r2] (unnormalized), on gpsimd -----
            D = pool.tile([128, 256], F32)    # [pl][t 8][q 4][b 8]
            Dv = D[:, :].rearrange("p (t q b) -> p q t b", q=4, b=8)
            G.tensor_scalar(out=Dv[:, 0, :, :], in0=PQv[:, :, 2, :],
                            scalar1=0.0, op0=ALU.max, scalar2=None)  # w
            d12 = D[:, :].rearrange("p (t q b) -> p t q b", q=4, b=8)[:, :, 1:3, :]
            cxy12 = CXY[:, :].rearrange("p (t c b) -> p t c b", t=8, c=2, b=8)
            wb2 = Dv[:, 0, :, :].unsqueeze(2).broadcast_to([128, 8, 2, 8])
            G.tensor_tensor(out=d12, in0=cxy12, in1=wb2, op=ALU.mult)
            G.tensor_tensor(out=Dv[:, 3, :, :],
                            in0=R2[:, :].rearrange("p (t b) -> p t b", t=8),
                            in1=Dv[:, 0, :, :], op=ALU.mult)

            # ---------------- moments: M = C^T D ----------------
            psMom = psp.tile([6, 32], F32, tag="psm")
            for t in range(8):
                T.matmul(psMom[:, :], C6[:, 6 * t : 6 * t + 6],
                         D[:, 32 * t : 32 * t + 32], start=(t == 0), stop=(t == 7))
            Msb = pool.tile([6, 32], F32)
            V.tensor_copy(Msb[:, :], psMom[:, :])
            # normalize the moment columns: [wp, wq] *= 1/s ; [wr] *= 1/s^2
            psC6 = psp.tile([6, 24], F32, tag="pss")
            T.matmul(psC6[:, :], ONESR[0:1, 0:6], IR24[:, :], start=True,
                     stop=True)
            SC6 = pool.tile([6, 24], F32)
            V.tensor_copy(SC6[:, :], psC6[:, :])
            V.tensor_tensor(out=Msb[:, 8:32], in0=Msb[:, 8:32], in1=SC6[:, :],
                            op=ALU.mult)

            # ---------------- assemble [A^T A | A^T b] via PE ----------------
            psA = psp.tile([72, 8], F32, tag="psm")
            for qq in range(4):
                T.matmul(psA[:, :], EQ[:, 72 * qq : 72 * qq + 72],
                         Msb[0:6, 8 * qq : 8 * qq + 8], start=(qq == 0),
                         stop=(qq == 3))
            AS = pool.tile([72, 8], F32)
            V.tensor_copy(AS[:, :], psA[:, :])
            psAT = psp.tile([8, 72], F32, tag="pss")
            T.transpose(psAT[:, :], AS[:, :], IDN)
            AUG = pool.tile([8, 72], F32)
            V.tensor_copy(AUG[:, :], psAT[:, :])
            V.tensor_scalar(out=AUG[:, 0:71:10], in0=AUG[:, 0:71:10],
                            scalar1=EPS, op0=ALU.add, scalar2=None)

            # ---------------- per-batch scalars to partitions --------------
            PR = pool.tile([1, 128], F32)
            V.tensor_copy(PR[:, 0:8], MRow[:, 0:8])      # mx
            V.tensor_copy(PR[:, 32:40], MRow[:, 8:16])   # my
            V.tensor_copy(PR[:, 64:72], sRow[:, :])      # s_dst
            V.tensor_copy(PR[:, 96:104], MRow[:, 16:24])  # mean w (support)
            psSC = psp.tile([128, 1], F32, tag="psb")
            T.transpose(psSC[:, :], PR[:, :], IDN[0:1, 0:1])
            SC = pool.tile([128, 1], F32)
            V.tensor_copy(SC[:, :], psSC[:, :])
            SCC = pool.tile([8, 4], F32)
            V.tensor_copy(SCC[:, 0:1], SC[0:8, :])      # mx (dst mean x)
            V.tensor_copy(SCC[:, 1:2], SC[32:40, :])    # my
            V.tensor_copy(SCC[:, 2:3], SC[64:72, :])    # s_dst
            V.tensor_scalar(out=SCC[:, 3:4], in0=SC[96:104, :],
                            scalar1=1e-4, op0=ALU.is_gt, scalar2=None)  # gate

            # ---------------- Gauss-Jordan ----------------
            RK = pool.tile([8, 9], F32)
            PIV = pool.tile([8, 1], F32)
            U8 = pool.tile([8, 72], F32)
            for k in range(8):
                w_ = 9 - k  # active columns k..8
                V.reciprocal(PIV[:, :], AUG[:, 9 * k + k : 9 * k + k + 1])
                V.tensor_scalar(out=RK[:, 0:w_], in0=AUG[:, 9 * k + k : 9 * k + 9],
                                scalar1=PIV[:, :], op0=ALU.mult, scalar2=None)
                fcol = AUG[:, k : 72 : 9].unsqueeze(2).broadcast_to([8, 8, w_])
                rkb = RK[:, 0:w_].unsqueeze(1).broadcast_to([8, 8, w_])
                ucols = U8[:, :].rearrange("p (r c) -> p r c", r=8)[:, :, 0:w_]
                acols = AUG[:, :].rearrange("p (r c) -> p r c", r=8)[:, :, k:9]
                V.tensor_tensor(out=ucols, in0=fcol, in1=rkb, op=ALU.mult)
                V.tensor_tensor(out=acols, in0=acols, in1=ucols, op=ALU.subtract)
                V.tensor_copy(AUG[:, 9 * k + k : 9 * k + 9], RK[:, 0:w_])

            # ---------------- denormalize + gate ----------------
            c_ = V.tensor_copy
            HN = pool.tile([8, 9], F32)
            c_(HN[:, 0:8], AUG[:, 8:72:9])
            V.memset(HN[:, 8:9], 1.0)
            mx_sc, my_sc = SCC[:, 0:1], SCC[:, 1:2]
            s_sc, g_sc = SCC[:, 2:3], SCC[:, 3:4]
            T1 = pool.tile([8, 9], F32)
            H1 = pool.tile([8, 9], F32)
            V.tensor_scalar(out=T1[:, 0:3], in0=HN[:, 0:3], scalar1=s_sc,
                            op0=ALU.mult, scalar2=None)
            V.scalar_tensor_tensor(out=H1[:, 0:3], in0=HN[:, 6:9], scalar=mx_sc,
                                   in1=T1[:, 0:3], op0=ALU.mult, op1=ALU.add)
            V.tensor_scalar(out=T1[:, 3:6], in0=HN[:, 3:6], scalar1=s_sc,
                            op0=ALU.mult, scalar2=None)
            V.scalar_tensor_tensor(out=H1[:, 3:6], in0=HN[:, 6:9], scalar=my_sc,
                                   in1=T1[:, 3:6], op0=ALU.mult, op1=ALU.add)
            c_(H1[:, 6:9], HN[:, 6:9])
            H2 = pool.tile([8, 9], F32)
            H1v = H1[:, :].rearrange("p (r c) -> p r c", r=3)
            H2v = H2[:, :].rearrange("p (r c) -> p r c", r=3)
            V.tensor_scalar(out=H2v[:, :, 0:2], in0=H1v[:, :, 0:2],
                            scalar1=cc.a_ts, op0=ALU.mult, scalar2=None)
            T2 = pool.tile([8, 3], F32)
            T3 = pool.tile([8, 3], F32)
            V.tensor_scalar(out=T2[:, :], in0=H1[:, 0:9:3], scalar1=cc.c_ts,
                            op0=ALU.mult, scalar2=None)
            V.scalar_tensor_tensor(out=T3[:, :], in0=H1[:, 1:9:3], scalar=cc.d_ts,
                                   in1=T2[:, :], op0=ALU.mult, op1=ALU.add)
            V.tensor_tensor(out=H2[:, 2:9:3], in0=T3[:, :], in1=H1[:, 2:9:3],
                            op=ALU.add)
            ABSD = pool.tile([8, 1], F32)
            SGN = pool.tile([8, 1], F32)
            DEN = pool.tile([8, 1], F32)
            RECD = pool.tile([8, 1], F32)
            A.activation(ABSD[:, :], H2[:, 8:9], ACTF.Abs)
            V.tensor_scalar(out=ABSD[:, :], in0=ABSD[:, :], scalar1=1e-8,
                            op0=ALU.max, scalar2=None)
            V.tensor_scalar(out=SGN[:, :], in0=H2[:, 8:9], scalar1=0.0,
                            op0=ALU.is_lt, scalar2=-2.0, op1=ALU.mult)
            V.tensor_scalar(out=SGN[:, :], in0=SGN[:, :], scalar1=1.0,
                            op0=ALU.add, scalar2=None)
            V.tensor_tensor(out=DEN[:, :], in0=ABSD[:, :], in1=SGN[:, :],
                            op=ALU.mult)
            V.reciprocal(RECD[:, :], DEN[:, :])
            V.tensor_scalar(out=H2[:, :], in0=H2[:, :], scalar1=RECD[:, :],
                            op0=ALU.mult, scalar2=None)
            IG = pool.tile([8, 1], F32)
            TI = pool.tile([8, 9], F32)
            OUTt = pool.tile([8, 9], F32)
            V.tensor_scalar(out=IG[:, :], in0=g_sc, scalar1=-1.0, op0=ALU.mult,
                            scalar2=1.0, op1=ALU.add)
            V.tensor_scalar(out=TI[:, :], in0=IEYE[:, :], scalar1=IG[:, :],
                            op0=ALU.mult, scalar2=None)
            V.scalar_tensor_tensor(out=OUTt[:, :], in0=H2[:, :], scalar=g_sc,
                                   in1=TI[:, :], op0=ALU.mult, op1=ALU.add)
            S.dma_start(Hout.ap().rearrange("b r c -> b (r c)"), OUTt[:, :])

    nc.compile()
    return nc


# ---------------------------------------------------------------------------
# host wrapper
# ---------------------------------------------------------------------------

_CACHE = {}


def _get(img_h, img_w):
    key = (int(img_h), int(img_w))
    if key not in _CACHE:
        cc = _Consts(*key)
        _CACHE[key] = (cc, _build_program(cc))
    return _CACHE[key]


def _in_maps(cc, flow, mask):
    flow = np.ascontiguousarray(flow, np.float32)
    mask = np.ascontiguousarray(mask, np.float32)
    fm = np.concatenate([flow, mask], axis=1)  # (64, 3, H, W)
    maps = []
    for c in range(NCORES):
        shard = fm[c * BPC : (c + 1) * BPC].transpose(1, 0, 2, 3)  # (3,8,H,W)
        maps.append({
            "fm": np.ascontiguousarray(shard),
            "CA": cc.CA, "CB": cc.CB,
        })
    return maps


def run(flow, mask, img_h, img_w, trace=False, **spmd_kwargs):
    cc, nc = _get(img_h, img_w)
    res = bass_utils.run_bass_kernel_spmd(
        nc, _in_maps(cc, flow, mask), list(range(NCORES)), trace=trace,
        **spmd_kwargs
    )
    out = np.concatenate([res.results[c]["H"] for c in range(NCORES)], axis=0)
    return out.astype(np.float32), res


def kernel(flow, mask, img_h, img_w):
    out, _ = run(flow, mask, img_h, img_w)
    return out


# revision 18
# speedup vs baseline: 1.2386x; 1.2386x over previous
"""Trainium2 Bass kernel for DifferentiableDLT (batched weighted-DLT homography fit).

Contract: kernel(**inputs) takes FULL inputs
    flow (64, 2, 320, 576) f32, mask (64, 1, 320, 576) f32, img_h, img_w
and returns the FULL output (64, 3, 3) f32.

Strategy (pure data parallel, 8 batches/core x 8 cores):
  The 1024 sample points form a fixed separable 32x32 grid, so bilinear
  sampling touches only 32 pair-rows (y0, y0+1) per image, and the y-steps
  follow a 9,9,9,10 pattern whose irregular step always lands on a k%4
  boundary.  Host-side we concatenate flow and mask into one (8,3,H,W)
  tensor so a single pipeline handles both.  Per core:
    1. 9 direct strided HWDGE dma_start pushes (no gather / index tables /
       gpsimd descriptor generation) land all 32 pair-rows x 24 images into
       SBUF as tFM[96=(kd4,c3,b8)][s8][1152], k = 4*s + kd.
    2. Column-select the 64 needed columns per pair-row with 9 uniform-
       stride-run copies per (half, c2), split across Vector and GpSimd.
    3. Bilinear lerp in y then x (broadcast weight tiles), per s-half so
       compute overlaps the tail of the DMA.
    4. Per half, one PE matmul transposes points onto partitions, fused
       with image-coord scaling (diag) and grid offsets (rank-5 factor).
    5. Hartley-normalize dst points; build weighted feature products D =
       [w, w*p, w*q, w*r2]; moments M = C^T @ D via 8 accumulating matmuls.
    6. Assemble the 8x9 augmented normal equations per batch (batch on
       partitions) and solve with unpivoted Gauss-Jordan (SPD + eps*I).
    7. Denormalize H, sign/scale fix, support gate, DMA out (8,3,3).
"""

import dataclasses
import math
import numpy as np

import concourse.bass as bass
import concourse.bacc as bacc
import concourse.mybir as mybir
from concourse import tile, library_config
from concourse import bass_utils

F32 = mybir.dt.float32
ALU = mybir.AluOpType
ACTF = mybir.ActivationFunctionType

NCORES = 8
BPC = 8          # batches per core
HF, WF = 320, 576
NG = 32          # grid is NG x NG points
NPTS = NG * NG
EPS = 1e-6
NC_IM = 3        # flow-x, flow-y, mask channels in the fused input
NIM = NC_IM * BPC   # 24 images per core
PW = 2 * WF      # pair-row width (rows y0, y0+1 contiguous)

# column layout offsets inside the packed constant tensors
CA_WY, CA_WX, CA_W = 0, 8, 40
CB_SXYD, CB_G5, CB_GR5, CB_C6, CB_IDN, CB_EQ, CB_W = 0, 96, 224, 320, 368, 440, 728

# ---------------------------------------------------------------------------
# host-side constant computation
# ---------------------------------------------------------------------------


def _grid_1d(size, n):
    m = int(size * 0.05)
    return np.linspace(m, size - m - 1, n, dtype=np.float32)


def _segments(x0):
    """Maximal uniform-step segments (start, len, step) covering x0."""
    segs = []
    i = 0
    n = len(x0)
    while i < n:
        if i == n - 1:
            segs.append((i, 1, 1))
            break
        st = x0[i + 1] - x0[i]
        j = i + 1
        while j + 1 < n and x0[j + 1] - x0[j] == st:
            j += 1
        segs.append((i, j - i + 1, int(st)))
        i = j + 1
    return segs


class _Consts:
    def __init__(self, img_h, img_w):
        ys = _grid_1d(HF, NG)
        xs = _grid_1d(WF, NG)
        y0 = np.floor(ys).astype(np.int64)
        x0 = np.floor(xs).astype(np.int64)
        wy = ys.astype(np.float64) - y0
        wx = xs.astype(np.float64) - x0
        self.segs = _segments(x0)
        self.x0 = x0
        self.y0 = y0
        self.sx = float(np.float32((img_w - 1) / max(WF - 1, 1)))
        self.sy = float(np.float32((img_h - 1) / max(HF - 1, 1)))

        # ---- row-push plan: k = 4*s + kd.  A full slot with uniform y-delta
        # is one push (24-image dim outermost so the DMA sprays 16 engines);
        # an irregular slot falls back to per-k pushes (stride-4 partitions).
        self.pushes = []  # (k0, nk, s, delta)
        for s in range(8):
            ks = y0[4 * s : 4 * s + 4]
            d = np.diff(ks)
            if (d == d[0]).all():
                self.pushes.append((4 * s, 4, s, int(d[0])))
            else:
                for kd in range(4):
                    self.pushes.append((4 * s + kd, 1, s, 0))

        # constant Hartley normalization of the source points (image coords)
        jj = np.arange(NPTS) // NG
        ii = np.arange(NPTS) % NG
        sxi = xs.astype(np.float64)[ii] * self.sx
        syi = ys.astype(np.float64)[jj] * self.sy
        mx, my = sxi.mean(), syi.mean()
        cxs, cys = sxi - mx, syi - my
        s_src = max(np.sqrt(cxs * cxs + cys * cys).mean() / math.sqrt(2.0), 1e-8)
        # per x/y 1-D normalized coords
        u1 = (xs.astype(np.float64) * self.sx - mx) / s_src   # (32,) by i
        v1 = (ys.astype(np.float64) * self.sy - my) / s_src   # (32,) by k
        # T_src = [[1/s,0,-mx/s],[0,1/s,-my/s],[0,0,1]] immediates
        self.a_ts = float(np.float32(1.0 / s_src))
        self.c_ts = float(np.float32(-mx / s_src))
        self.d_ts = float(np.float32(-my / s_src))

        # partition p (96) = (c*8 + b)*4 + kd = c*32 + b*4 + kd
        pp = np.arange(96)
        kd_p = pp % 4
        c_p = pp // 32

        # ---- CONSTA [128, 40]: WY [96,8] (wy[4s+kd]), WX [96,32] (wx[i])
        CA = np.zeros((128, CA_W), np.float32)
        CA[:96, 0:8] = wy[4 * np.arange(8)[None, :] + kd_p[:, None]]
        CA[:96, 8:40] = np.tile(wx[None, :], (96, 1))
        self.CA = CA

        # ---- CONSTB [128, CB_W] ----
        CB = np.zeros((128, CB_W), np.float32)
        # SXYD [96,96] diag: c==0 -> sx, c==1 -> sy, c==2 -> 1
        sxy = np.where(c_p == 0, self.sx, np.where(c_p == 1, self.sy, 1.0))
        CB[:96, CB_SXYD : CB_SXYD + 96] = np.eye(96) * sxy[None, :]
        # G5 [5,128]: rows sp: (pl//32==sp); row4: xs[pl%32]
        # duplicated at partition 32 (base partition must match GR5's h block)
        pl = np.arange(128)
        G5 = np.zeros((5, 128), np.float64)
        for sp in range(4):
            G5[sp] = (pl // 32 == sp).astype(np.float64)
        G5[4] = xs.astype(np.float64)[pl % 32]
        CB[:5, CB_G5 : CB_G5 + 128] = G5.astype(np.float32)
        CB[32:37, CB_G5 : CB_G5 + 128] = G5.astype(np.float32)
        # GR5 rows 32h+sp: (c==1)*ys[4*(4h+sp)+kd]*sy ; 32h+4: (c==0)*sx
        # (h=1 block lives at partition 32 -- matmul base-partition rule)
        GR5 = np.zeros((37, 96), np.float64)
        for h in range(2):
            for sp in range(4):
                kk = 4 * (4 * h + sp) + kd_p
                GR5[32 * h + sp] = np.where(
                    c_p == 1, ys.astype(np.float64)[kk] * self.sy, 0.0)
            GR5[32 * h + 4] = np.where(c_p == 0, self.sx, 0.0)
        CB[:37, CB_GR5 : CB_GR5 + 96] = GR5.astype(np.float32)
        # C6 [128, 48]: feats [uu, uv, u, vv, v, 1] at (pl=(s',i), t=(h,kd))
        C6 = np.zeros((128, 48), np.float64)
        spl = pl // 32
        ipl = pl % 32
        for t in range(8):
            h, kd = t // 4, t % 4
            kk = 4 * (4 * h + spl) + kd
            u = u1[ipl]
            v = v1[kk]
            C6[:, 6 * t : 6 * t + 6] = np.stack(
                [u * u, u * v, u, v * v, v, np.ones_like(u)], -1)
        CB[:, CB_C6 : CB_C6 + 48] = C6.astype(np.float32)
        # IDN72
        CB[:72, CB_IDN : CB_IDN + 72] = np.eye(72, dtype=np.float32)
        # EQ [6, 288]
        E = np.zeros((4, 6, 72), np.float64)
        sym = [[0, 1, 2], [1, 3, 4], [2, 4, 5]]
        for r in range(3):
            for c in range(3):
                m = sym[r][c]
                E[0, m, r * 9 + c] += 1
                E[0, m, (r + 3) * 9 + (c + 3)] += 1
        cr = [[0, 1], [1, 3], [2, 4]]
        for q, r0 in ((1, 0), (2, 3)):
            for r in range(3):
                for c2 in range(2):
                    m = cr[r][c2]
                    E[q, m, (r0 + r) * 9 + 6 + c2] += -1
                    E[q, m, (6 + c2) * 9 + (r0 + r)] += -1
            for r, m in ((0, 2), (1, 4), (2, 5)):
                E[q, m, (r0 + r) * 9 + 8] += 1
        rb = [[0, 1], [1, 3]]
        for r in range(2):
            for c2 in range(2):
                E[3, rb[r][c2], (6 + r) * 9 + 6 + c2] += 1
        E[3, 2, 6 * 9 + 8] += -1
        E[3, 4, 7 * 9 + 8] += -1
        CB[:6, CB_EQ : CB_EQ + 288] = np.ascontiguousarray(
            E.transpose(1, 0, 2).reshape(6, 288)).astype(np.float32)
        self.CB = CB


# ---------------------------------------------------------------------------
# device program
# ---------------------------------------------------------------------------


def _build_program(cc: _Consts):
    nc = bacc.Bacc("TRN2", target_bir_lowering=False, debug=False)

    fm = nc.dram_tensor("fm", [NC_IM, BPC, HF, WF], F32, kind="ExternalInput")
    CAd = nc.dram_tensor("CA", [128, CA_W], F32, kind="ExternalInput")
    CBd = nc.dram_tensor("CB", [128, CB_W], F32, kind="ExternalInput")
    Hout = nc.dram_tensor("H", [BPC, 3, 3], F32, kind="ExternalOutput")

    V = nc.vector
    A = nc.scalar
    T = nc.tensor
    G = nc.gpsimd
    S = nc.sync

    fm_flat = fm.ap().rearrange("c b h w -> (c b h w)").unsqueeze(0)

    def push_src(k0, nk, delta):
        # (b,c)-image dim outermost: DMA sprays descriptors over 16 engines
        off = int(cc.y0[k0]) * WF
        v = fm_flat[:, off : off + 1]
        if nk > 1:
            ap = [[HF * WF, NIM], [delta * WF, nk], [1, PW]]
        else:
            ap = [[HF * WF, NIM], [1, PW]]
        return dataclasses.replace(v, ap=ap)

    with tile.TileContext(nc) as tc:
        with (
            tc.tile_pool(name="sb", bufs=1) as pool,
            tc.tile_pool(name="ps", bufs=1, space="PSUM") as psp,
        ):
            tFM = pool.tile([96, 8, PW], F32)     # [(kd,c,b)][s][pair row]
            CAt = pool.tile([128, CA_W], F32, tag="CA")
            CBt = pool.tile([128, CB_W], F32, tag="CB")

            # ---------------- row pushes + constants (2 HWDGE queues) ------
            def dst_slice(k0, nk, s):
                if nk == 4:       # full slot: partitions 0..95 contiguous
                    return tFM[0:96, s : s + 1, :].rearrange("p o w -> p (o w)")
                kd = k0 % 4       # single k: every 4th partition
                return tFM[kd : 96 : 4, s : s + 1, :].rearrange(
                    "p o w -> p (o w)")

            # order: slot0 | CONSTA | slot1..: alternate queues
            q = [S, A]
            S.dma_start(dst_slice(cc.pushes[0][0], cc.pushes[0][1],
                                  cc.pushes[0][2]),
                        push_src(cc.pushes[0][0], cc.pushes[0][1],
                                 cc.pushes[0][3]))
            A.dma_start(CAt[:, :], CAd[:])
            qi = 0
            for n, (k0, nk, s, d) in enumerate(cc.pushes[1:], start=1):
                q[qi].dma_start(dst_slice(k0, nk, s), push_src(k0, nk, d))
                qi ^= 1
                if n == 3:
                    q[qi].dma_start(CBt[:, :], CBd[:])
                    qi ^= 1

            # small on-chip constants
            IEYE = pool.tile([8, 9], F32, tag="IEYE")
            V.memset(IEYE[:, :], 0.0)
            V.memset(IEYE[:, 0:9:4], 1.0)
            ONESC = pool.tile([128, 1], F32, tag="ONESC")
            V.memset(ONESC[:, :], 1.0 / NPTS)
            ONESR = pool.tile([1, 128], F32, tag="ONESR")
            V.memset(ONESR[:, :], 1.0)
            # prefetch the ACT function table (Sqrt/Abs) off the critical path
            ACTJ = pool.tile([8, 2], F32, tag="ACTJ")
            V.memset(ACTJ[:, :], 1.0)
            A.activation(ACTJ[:, 0:1], ACTJ[:, 1:2], ACTF.Sqrt)
            A.activation(ACTJ[:, 1:2], ACTJ[:, 0:1], ACTF.Abs)

            WY = CAt[0:96, CA_WY : CA_WY + 8]
            WX = CAt[0:96, CA_WX : CA_WX + 32]
            SXYD = CBt[0:96, CB_SXYD : CB_SXYD + 96]
            G5GR = CBt[0:37, :]  # G5/GR5 blocks at partitions 32h
            C6 = CBt[0:128, CB_C6 : CB_C6 + 48]
            IDN = CBt[0:72, CB_IDN : CB_IDN + 72]
            EQ = CBt[0:6, CB_EQ : CB_EQ + 288]

            # ------- column select + bilinear interp, per s-half -------
            GxFM = pool.tile([96, 2, 8, 32, 2], F32)  # [a][s][i][c2]
            VF = pool.tile([96, 8, 32, 2], F32)
            DV = pool.tile([96, 8, 32, 2], F32)
            DX = pool.tile([96, 256], F32)
            sampFM = pool.tile([96, 256], F32)        # [(kd,c,b)][(s,i)]
            psF = psp.tile([128, 192], F32)
            PQs = pool.tile([128, 192], F32)          # [pl=(s',i)][(t,c,b)]
            tFMv = tFM[:, :, :].rearrange("p s (a w) -> p a s w", a=2)

            for h in range(2):
                sl = slice(4 * h, 4 * h + 4)
                eng = 0
                for (i0, L, st) in cc.segs:
                    base = int(cc.x0[i0])
                    for c2 in (0, 1):
                        src_ = tFMv[:, :, sl,
                                    base + c2 : base + c2 + (L - 1) * st + 1 : st]
                        dst = GxFM[:, :, sl, i0 : i0 + L, c2]
                        (V if eng == 0 else G).tensor_copy(dst, src_)
                        eng ^= 1
                # y lerp: V = G0 + (G1-G0)*wy  on [96, (s4,i,c2)=256]
                g0 = GxFM[:, 0, sl, :, :]
                g1 = GxFM[:, 1, sl, :, :]
                dv = DV[:, sl, :, :]
                vf = VF[:, sl, :, :]
                wyb = WY[:, sl].unsqueeze(2).unsqueeze(3).broadcast_to(
                    [96, 4, 32, 2])
                V.tensor_tensor(out=dv, in0=g1, in1=g0, op=ALU.subtract)
                V.tensor_tensor(out=dv, in0=dv, in1=wyb, op=ALU.mult)
                V.tensor_tensor(out=vf, in0=dv, in1=g0, op=ALU.add)
                # x lerp: samp = V0 + (V1-V0)*wx  on [96, (s4,i)=128]
                vv = VF[:, sl, :, :]
                dx = DX[:, 128 * h : 128 * h + 128].rearrange(
                    "p (s i) -> p s i", s=4)
                sm = sampFM[:, 128 * h : 128 * h + 128].rearrange(
                    "p (s i) -> p s i", s=4)
                wxb = WX[:, :].unsqueeze(1).broadcast_to([96, 4, 32])
                V.tensor_tensor(out=dx, in0=vv[:, :, :, 1], in1=vv[:, :, :, 0],
                                op=ALU.subtract)
                V.tensor_tensor(out=dx, in0=dx, in1=wxb, op=ALU.mult)
                V.tensor_tensor(out=sm, in0=dx, in1=vv[:, :, :, 0], op=ALU.add)

                # transpose to points-on-partitions, fused with image-coord
                # scaling and grid offsets
                T.matmul(psF[:, 96 * h : 96 * h + 96],
                         sampFM[:, 128 * h : 128 * h + 128], SXYD,
                         start=True, stop=False)
                T.matmul(psF[:, 96 * h : 96 * h + 96],
                         G5GR[32 * h : 32 * h + 5, CB_G5 : CB_G5 + 128],
                         G5GR[32 * h : 32 * h + 5, CB_GR5 : CB_GR5 + 96],
                         start=False, stop=True)
                V.tensor_copy(PQs[:, 96 * h : 96 * h + 96],
                              psF[:, 96 * h : 96 * h + 96])

            # ---------------- Hartley stats ----------------
            # PQs free layout: (h2, c3, b8, kd4)
            psSum = psp.tile([1, 192], F32, tag="pss")
            T.matmul(psSum[:, :], ONESC[:, :], PQs[:, :], start=True, stop=True)
            SRow = pool.tile([1, 192], F32)
            V.tensor_copy(SRow[:, :], psSum[:, :])
            HRow = pool.tile([1, 96], F32)
            V.tensor_tensor(out=HRow[:, :], in0=SRow[:, 0:96],
                            in1=SRow[:, 96:192], op=ALU.add)
            MRow = pool.tile([1, 24], F32)   # [(c3,b8)] means over points
            V.tensor_reduce(
                out=MRow[:, :].unsqueeze(1),
                in_=HRow[:, :].rearrange("o (cb k) -> o cb k", k=4),
                axis=mybir.AxisListType.X,
                op=ALU.add,
            )
            psMB = psp.tile([128, 24], F32, tag="psb")
            T.matmul(psMB[:, :], ONESR[:, :], MRow[:, :], start=True, stop=True)
            MB = pool.tile([128, 24], F32)
            V.tensor_copy(MB[:, :], psMB[:, :])

            CXY = pool.tile([128, 128], F32)  # centered dst [(h,c2,b,kd)]
            SQ = pool.tile([128, 128], F32)
            R2 = pool.tile([128, 64], F32)    # [(h,b,kd)]
            SQR = pool.tile([128, 64], F32)
            mbv = MB[:, 0:16].rearrange("p (c b) -> p c b", c=2).unsqueeze(
                3).broadcast_to([128, 2, 8, 4])
            for h in range(2):
                V.tensor_tensor(
                    out=CXY[:, 64 * h : 64 * h + 64].rearrange(
                        "p (c b k) -> p c b k", c=2, b=8),
                    in0=PQs[:, 96 * h : 96 * h + 64].rearrange(
                        "p (c b k) -> p c b k", c=2, b=8),
                    in1=mbv, op=ALU.subtract)
            V.tensor_tensor(out=SQ[:, :], in0=CXY[:, :], in1=CXY[:, :],
                            op=ALU.mult)
            sq3 = SQ[:, :].rearrange("p (h c m) -> p h c m", h=2, c=2)
            V.tensor_tensor(out=R2[:, :].rearrange("p (h m) -> p h m", h=2),
                            in0=sq3[:, :, 0, :], in1=sq3[:, :, 1, :], op=ALU.add)
            A.activation(SQR[:, :], R2[:, :], ACTF.Sqrt)
            psSq = psp.tile([1, 64], F32, tag="pss")
            T.matmul(psSq[:, :], ONESC[:, :], SQR[:, :], start=True, stop=True)
            SqRow = pool.tile([1, 64], F32)
            V.tensor_copy(SqRow[:, :], psSq[:, :])
            HSq = pool.tile([1, 32], F32)
            V.tensor_tensor(out=HSq[:, :], in0=SqRow[:, 0:32],
                            in1=SqRow[:, 32:64], op=ALU.add)
            sRow = pool.tile([1, 8], F32)
            V.tensor_reduce(
                out=sRow[:, :].unsqueeze(1),
                in_=HSq[:, :].rearrange("o (b k) -> o b k", k=4),
                axis=mybir.AxisListType.X,
                op=ALU.add,
            )
            V.tensor_scalar(out=sRow[:, :], in0=sRow[:, :],
                            scalar1=1.0 / math.sqrt(2.0), op0=ALU.mult,
                            scalar2=1e-8, op1=ALU.max)
            IR24 = pool.tile([1, 24], F32)   # [inv | inv | inv^2]
            V.reciprocal(IR24[:, 0:8], sRow[:, :])
            V.tensor_copy(IR24[:, 8:16], IR24[:, 0:8])
            V.tensor_tensor(out=IR24[:, 16:24], in0=IR24[:, 0:8],
                            in1=IR24[:, 0:8], op=ALU.mult)

            # ------ D = [w, w*cx, w*cy, w*r2] (unnormalized), on gpsimd -----
            D = pool.tile([128, 256], F32)    # [pl][(h,kd,q4,b8)] = t-blocked
            for h in range(2):
                Dh = D[:, 128 * h : 128 * h + 128].rearrange(
                    "p (kd q b) -> p kd q b", kd=4, q=4)
                wd = Dh[:, :, 0, :]                               # [128,4,8]
                G.tensor_scalar(
                    out=wd,
                    in0=PQs[:, 96 * h + 64 : 96 * h + 96].rearrange(
                        "p (b k) -> p k b", b=8),
                    scalar1=0.0, op0=ALU.max, scalar2=None)       # w
                G.tensor_tensor(
                    out=Dh[:, :, 1:3, :],
                    in0=CXY[:, 64 * h : 64 * h + 64].rearrange(
                        "p (c b k) -> p k c b", c=2, b=8),
                    in1=wd.unsqueeze(2).broadcast_to([128, 4, 2, 8]),
                    op=ALU.mult)
                G.tensor_tensor(
                    out=Dh[:, :, 3, :],
                    in0=R2[:, 32 * h : 32 * h + 32].rearrange(
                        "p (b k) -> p k b", b=8),
                    in1=wd, op=ALU.mult)

            # ---------------- moments: M = C^T D ----------------
            psMom = psp.tile([6, 32], F32, tag="psm")
            for t in range(8):
                T.matmul(psMom[:, :], C6[:, 6 * t : 6 * t + 6],
                         D[:, 32 * t : 32 * t + 32], start=(t == 0), stop=(t == 7))
            Msb = pool.tile([6, 32], F32)
            V.tensor_copy(Msb[:, :], psMom[:, :])
            # normalize the moment columns: [wp, wq] *= 1/s ; [wr] *= 1/s^2
            psC6 = psp.tile([6, 24], F32, tag="pss")
            T.matmul(psC6[:, :], ONESR[0:1, 0:6], IR24[:, :], start=True,
                     stop=True)
            SC6 = pool.tile([6, 24], F32)
            V.tensor_copy(SC6[:, :], psC6[:, :])
            V.tensor_tensor(out=Msb[:, 8:32], in0=Msb[:, 8:32], in1=SC6[:, :],
                            op=ALU.mult)

            # ---------------- assemble [A^T A | A^T b] via PE ----------------
            psA = psp.tile([72, 8], F32, tag="psm")
            for qq in range(4):
                T.matmul(psA[:, :], EQ[:, 72 * qq : 72 * qq + 72],
                         Msb[0:6, 8 * qq : 8 * qq + 8], start=(qq == 0),
                         stop=(qq == 3))
            AS = pool.tile([72, 8], F32)
            V.tensor_copy(AS[:, :], psA[:, :])
            psAT = psp.tile([8, 72], F32, tag="pss")
            T.transpose(psAT[:, :], AS[:, :], IDN)
            AUG = pool.tile([8, 72], F32)
            V.tensor_copy(AUG[:, :], psAT[:, :])
            V.tensor_scalar(out=AUG[:, 0:71:10], in0=AUG[:, 0:71:10],
                            scalar1=EPS, op0=ALU.add, scalar2=None)

            # ---------------- per-batch scalars to partitions --------------
            PR = pool.tile([1, 128], F32)
            V.tensor_copy(PR[:, 0:8], MRow[:, 0:8])      # mx
            V.tensor_copy(PR[:, 32:40], MRow[:, 8:16])   # my
            V.tensor_copy(PR[:, 64:72], sRow[:, :])      # s_dst
            V.tensor_copy(PR[:, 96:104], MRow[:, 16:24])  # mean w (support)
            psSC = psp.tile([128, 1], F32, tag="psb")
            T.transpose(psSC[:, :], PR[:, :], IDN[0:1, 0:1])
            SC = pool.tile([128, 1], F32)
            V.tensor_copy(SC[:, :], psSC[:, :])
            SCC = pool.tile([8, 4], F32)
            V.tensor_copy(SCC[:, 0:1], SC[0:8, :])      # mx (dst mean x)
            V.tensor_copy(SCC[:, 1:2], SC[32:40, :])    # my
            V.tensor_copy(SCC[:, 2:3], SC[64:72, :])    # s_dst
            V.tensor_scalar(out=SCC[:, 3:4], in0=SC[96:104, :],
                            scalar1=1e-4, op0=ALU.is_gt, scalar2=None)  # gate

            # ---------------- Gauss-Jordan ----------------
            RK = pool.tile([8, 9], F32)
            PIV = pool.tile([8, 1], F32)
            U8 = pool.tile([8, 72], F32)
            for k in range(8):
                w_ = 9 - k  # active columns k..8
                V.reciprocal(PIV[:, :], AUG[:, 9 * k + k : 9 * k + k + 1])
                V.tensor_scalar(out=RK[:, 0:w_], in0=AUG[:, 9 * k + k : 9 * k + 9],
                                scalar1=PIV[:, :], op0=ALU.mult, scalar2=None)
                fcol = AUG[:, k : 72 : 9].unsqueeze(2).broadcast_to([8, 8, w_])
                rkb = RK[:, 0:w_].unsqueeze(1).broadcast_to([8, 8, w_])
                ucols = U8[:, :].rearrange("p (r c) -> p r c", r=8)[:, :, 0:w_]
                acols = AUG[:, :].rearrange("p (r c) -> p r c", r=8)[:, :, k:9]
                V.tensor_tensor(out=ucols, in0=fcol, in1=rkb, op=ALU.mult)
                V.tensor_tensor(out=acols, in0=acols, in1=ucols, op=ALU.subtract)
                V.tensor_copy(AUG[:, 9 * k + k : 9 * k + 9], RK[:, 0:w_])

            # ---------------- denormalize + gate ----------------
            c_ = V.tensor_copy
            HN = pool.tile([8, 9], F32)
            c_(HN[:, 0:8], AUG[:, 8:72:9])
            V.memset(HN[:, 8:9], 1.0)
            mx_sc, my_sc = SCC[:, 0:1], SCC[:, 1:2]
            s_sc, g_sc = SCC[:, 2:3], SCC[:, 3:4]
            T1 = pool.tile([8, 9], F32)
            H1 = pool.tile([8, 9], F32)
            V.tensor_scalar(out=T1[:, 0:3], in0=HN[:, 0:3], scalar1=s_sc,
                            op0=ALU.mult, scalar2=None)
            V.scalar_tensor_tensor(out=H1[:, 0:3], in0=HN[:, 6:9], scalar=mx_sc,
                                   in1=T1[:, 0:3], op0=ALU.mult, op1=ALU.add)
            V.tensor_scalar(out=T1[:, 3:6], in0=HN[:, 3:6], scalar1=s_sc,
                            op0=ALU.mult, scalar2=None)
            V.scalar_tensor_tensor(out=H1[:, 3:6], in0=HN[:, 6:9], scalar=my_sc,
                                   in1=T1[:, 3:6], op0=ALU.mult, op1=ALU.add)
            c_(H1[:, 6:9], HN[:, 6:9])
            H2 = pool.tile([8, 9], F32)
            H1v = H1[:, :].rearrange("p (r c) -> p r c", r=3)
            H2v = H2[:, :].rearrange("p (r c) -> p r c", r=3)
            V.tensor_scalar(out=H2v[:, :, 0:2], in0=H1v[:, :, 0:2],
                            scalar1=cc.a_ts, op0=ALU.mult, scalar2=None)
            T2 = pool.tile([8, 3], F32)
            T3 = pool.tile([8, 3], F32)
            V.tensor_scalar(out=T2[:, :], in0=H1[:, 0:9:3], scalar1=cc.c_ts,
                            op0=ALU.mult, scalar2=None)
            V.scalar_tensor_tensor(out=T3[:, :], in0=H1[:, 1:9:3], scalar=cc.d_ts,
                                   in1=T2[:, :], op0=ALU.mult, op1=ALU.add)
            V.tensor_tensor(out=H2[:, 2:9:3], in0=T3[:, :], in1=H1[:, 2:9:3],
                            op=ALU.add)
            ABSD = pool.tile([8, 1], F32)
            SGN = pool.tile([8, 1], F32)
            DEN = pool.tile([8, 1], F32)
            RECD = pool.tile([8, 1], F32)
            A.activation(ABSD[:, :], H2[:, 8:9], ACTF.Abs)
            V.tensor_scalar(out=ABSD[:, :], in0=ABSD[:, :], scalar1=1e-8,
                            op0=ALU.max, scalar2=None)
            V.tensor_scalar(out=SGN[:, :], in0=H2[:, 8:9], scalar1=0.0,
                            op0=ALU.is_lt, scalar2=-2.0, op1=ALU.mult)
            V.tensor_scalar(out=SGN[:, :], in0=SGN[:, :], scalar1=1.0,
                            op0=ALU.add, scalar2=None)
            V.tensor_tensor(out=DEN[:, :], in0=ABSD[:, :], in1=SGN[:, :],
                            op=ALU.mult)
            V.reciprocal(RECD[:, :], DEN[:, :])
            V.tensor_scalar(out=H2[:, :], in0=H2[:, :], scalar1=RECD[:, :],
                            op0=ALU.mult, scalar2=None)
            IG = pool.tile([8, 1], F32)
            TI = pool.tile([8, 9], F32)
            OUTt = pool.tile([8, 9], F32)
            V.tensor_scalar(out=IG[:, :], in0=g_sc, scalar1=-1.0, op0=ALU.mult,
                            scalar2=1.0, op1=ALU.add)
            V.tensor_scalar(out=TI[:, :], in0=IEYE[:, :], scalar1=IG[:, :],
                            op0=ALU.mult, scalar2=None)
            V.scalar_tensor_tensor(out=OUTt[:, :], in0=H2[:, :], scalar=g_sc,
                                   in1=TI[:, :], op0=ALU.mult, op1=ALU.add)
            S.dma_start(Hout.ap().rearrange("b r c -> b (r c)"), OUTt[:, :])

    nc.compile()
    return nc


# ---------------------------------------------------------------------------
# host wrapper
# ---------------------------------------------------------------------------

_CACHE = {}


def _get(img_h, img_w):
    key = (int(img_h), int(img_w))
    if key not in _CACHE:
        cc = _Consts(*key)
        _CACHE[key] = (cc, _build_program(cc))
    return _CACHE[key]


def _in_maps(cc, flow, mask):
    flow = np.ascontiguousarray(flow, np.float32)
    mask = np.ascontiguousarray(mask, np.float32)
    fm = np.concatenate([flow, mask], axis=1)  # (64, 3, H, W)
    maps = []
    for c in range(NCORES):
        shard = fm[c * BPC : (c + 1) * BPC].transpose(1, 0, 2, 3)  # (3,8,H,W)
        maps.append({
            "fm": np.ascontiguousarray(shard),
            "CA": cc.CA, "CB": cc.CB,
        })
    return maps


def run(flow, mask, img_h, img_w, trace=False, **spmd_kwargs):
    cc, nc = _get(img_h, img_w)
    res = bass_utils.run_bass_kernel_spmd(
        nc, _in_maps(cc, flow, mask), list(range(NCORES)), trace=trace,
        **spmd_kwargs
    )
    out = np.concatenate([res.results[c]["H"] for c in range(NCORES)], axis=0)
    return out.astype(np.float32), res


def kernel(flow, mask, img_h, img_w):
    out, _ = run(flow, mask, img_h, img_w)
    return out
